# revision 25
# baseline (speedup 1.0000x reference)
"""Sliding-window (radius-8, K=17) single-head attention along W.

Full problem: feature/position [2, 128, 64, 256] f32; 1x1 convs Wq/Wk (+bias)
produce q/k; scores over a 17-wide window along W; softmax (zero-padded
windows contribute exp(0)=1 to the denominator); output is the attn-weighted
sum of windows of x = feature + position.

Sharding: data-parallel over (B, H) — the 128 (b, h) rows are independent;
each of the 8 cores gets 16 rows, two per iteration.

v2 design (all-f16 datapath, measured ~6e-3 rel err vs fp32 reference):
  x = f + p formed by DMA accumulate (SWDGE, gpsimd queue) — no engine adds.
  q = (Wq/sqrt(C)) x + bq/sqrt(C); k = Wk x + bk: f16 PE matmuls into f16
  PSUM, bias added during eviction (q on scalar+vector halves, k on scalar).
  Scores computed TRANSPOSED (keys on partitions) per 136-wide strip only
  (chunk1 keys 0..127 x queries 0..135, chunk2 keys 128..255 x queries
  120..255), f16 operands, f32 PSUM. A -30 band mask and the -6 softmax
  shift are ADDED pre-exp by an accumulating ident-matmul, so exp(S-6)
  stays in f16 range (max score ~13) and banned taps vanish (~1e-13).
  exp on scalar -> att f16 strips. den = ones^T @ att strips + ident^T @
  oob edge counts (only w<8 / w>247 have zero-pad taps; oob pre-scaled by
  e^-6). reciprocal_approx_fast per 4-row block; out = (xT @ att) * rden.
  xT comes from SBUF->SBUF DMA transposes (no PE transposes, no eviction).
  PE warm-up: dummy matmuls during the input-DMA ramp so the HAM clock
  gate reaches 2.4 GHz before real work (otherwise first ~3.4us run at
  1.2 GHz).

Schedule: software pipeline at 2-row iteration / 4-row super-iteration
granularity; input DMAs split f-blocks on the sync HWDGE queue and
p-accumulate blocks on the gpsimd SWDGE queue so issue overheads overlap.
"""

import numpy as np
from contextlib import ExitStack

import concourse.bacc as bacc
import concourse.mybir as mybir
import concourse.tile as tile
from concourse.ap import AP
from concourse.bass_utils import run_bass_kernel_spmd

# Enable the walrus ldw-opt pass (dedupes redundant LDWEIGHTS between
# consecutive matmuls sharing a stationary operand).
import concourse.bass_utils as _bu

if not getattr(_bu, "_ldwopt_patched", False):
    _orig_walrus_args = _bu.get_walrus_args

    def _walrus_args_ldwopt(arch, tmpdir, *, dve_root=None):
        args = _orig_walrus_args(arch, tmpdir, dve_root=dve_root)
        return [
            a.replace("--enable-ldw-opt=false", "--enable-ldw-opt=true")
            for a in args
        ]

    _bu.get_walrus_args = _walrus_args_ldwopt
    _bu._ldwopt_patched = True

B, C, H, W = 2, 128, 64, 256
R = 8
NCORES = 8
ROWS = B * H // NCORES        # 16 (b, h) rows per core
CORES_PER_B = NCORES // B     # 4
F32 = mybir.dt.float32
F16 = mybir.dt.float16
EXP = mybir.ActivationFunctionType.Exp
ADD = mybir.AluOpType.add
MULT = mybir.AluOpType.mult
SW = 136                      # strip width: chunk1 queries [0:136), chunk2 [120:256)
SHIFT = 6.0                   # exp(S - SHIFT) so att fits f16 (max S ~ 13.4)
CBW = 816                     # f16 const blob: wq|wk|ident|ones|mask(272)|oob(32)
WARMN = 25                    # PE warm-up dummy matmuls during the DMA ramp
# input blocks (row0, nrows): f/p on sync HWDGE, summed by gpsimd per block
BLOCKS = [(0, 2), (2, 2), (4, 4), (8, 4), (12, 4)]


def apn(t, dims, off=0):
    v = t[:]
    return AP(v.tensor, v.offset + off, list(v.ap[:1]) + list(dims))


def build_nc():
    nc = bacc.Bacc(trn_type="TRN2")
    f_ext = nc.dram_tensor("feature", [C, ROWS, W], F16, kind="ExternalInput")
    p_ext = nc.dram_tensor("position", [C, ROWS, W], F16, kind="ExternalInput")
    cb_ext = nc.dram_tensor("constb", [C, CBW], F16, kind="ExternalInput")
    cf_ext = nc.dram_tensor("constf", [C, 3], F32, kind="ExternalInput")
    out_ext = nc.dram_tensor("out", [C, ROWS, W], F16, kind="ExternalOutput")

    with tile.TileContext(nc) as tc, ExitStack() as ctx:
        const = ctx.enter_context(tc.tile_pool(name="const", bufs=1))
        xp = ctx.enter_context(tc.tile_pool(name="x", bufs=1))
        xtp = ctx.enter_context(tc.tile_pool(name="xt", bufs=1))
        qkp = ctx.enter_context(tc.tile_pool(name="qk", bufs=2))
        attp = ctx.enter_context(tc.tile_pool(name="att", bufs=4))
        rdp = ctx.enter_context(tc.tile_pool(name="rd", bufs=2))
        osp = ctx.enter_context(tc.tile_pool(name="os", bufs=2))
        psq = ctx.enter_context(tc.tile_pool(name="psq", bufs=1, space="PSUM"))
        psk = ctx.enter_context(tc.tile_pool(name="psk", bufs=1, space="PSUM"))
        pss = ctx.enter_context(tc.tile_pool(name="pss", bufs=1, space="PSUM"))
        psd = ctx.enter_context(tc.tile_pool(name="psd", bufs=1, space="PSUM"))
        pso = ctx.enter_context(tc.tile_pool(name="pso", bufs=1, space="PSUM"))

        # ---- warm-up seeds (no DMA deps) ----
        dm = const.tile([C, 128], F16, tag="dm")
        nc.vector.memset(dm[:], 0.125)
        wsb = const.tile([C, 1], F32, tag="wsb")
        # touch Exp once so the ACT table loads during the input-DMA ramp
        nc.scalar.activation(wsb[:], dm[:, 0:1], EXP)
        # touch a gpsimd tensor op so its library loads during the ramp too
        dmw = const.tile([C, 8], F16, tag="dmw")
        nc.gpsimd.tensor_tensor(dmw[:], dm[:, 0:8], dm[:, 8:16], ADD)

        # ---- input DMAs: f + p0/p1 on sync HWDGE, p2-p4 on scalar HWDGE ----
        fb, pb, xb = [], [], []
        for r0, nr in BLOCKS:
            fb.append(xp.tile([C, nr, W], F16, tag=f"f{r0}", name=f"f{r0}"))
            pb.append(xp.tile([C, nr, W], F16, tag=f"p{r0}", name=f"p{r0}"))
            xb.append(xp.tile([C, nr, W], F16, tag=f"x{r0}", name=f"x{r0}"))
        cb = const.tile([C, CBW], F16, tag="cb")
        cf = const.tile([C, 3], F32, tag="cf")
        # consts first: they gate the first matmuls / evictions
        nc.sync.dma_start(cb[:], cb_ext[:])
        nc.sync.dma_start(cf[:], cf_ext[:])
        nc.sync.dma_start(fb[0][:], f_ext[:, 0:2, :])
        nc.sync.dma_start(pb[0][:], p_ext[:, 0:2, :])
        nc.sync.dma_start(fb[1][:], f_ext[:, 2:4, :])
        nc.sync.dma_start(pb[1][:], p_ext[:, 2:4, :])
        for bi, (r0, nr) in enumerate(BLOCKS[2:], start=2):
            nc.sync.dma_start(fb[bi][:], f_ext[:, r0 : r0 + nr, :])
            nc.scalar.dma_start(pb[bi][:], p_ext[:, r0 : r0 + nr, :])
        # x = f + p: ramp blocks on the (idle) vector engine at 2x f16 rate,
        # the rest on gpsimd (block2 split across both for latency)
        nc.vector.tensor_tensor(xb[0][:], fb[0][:], pb[0][:], ADD)
        nc.vector.tensor_tensor(xb[1][:], fb[1][:], pb[1][:], ADD)
        nc.vector.tensor_tensor(
            xb[2][:, 0:2], fb[2][:, 0:2], pb[2][:, 0:2], ADD
        )
        nc.gpsimd.tensor_tensor(
            xb[2][:, 2:4], fb[2][:, 2:4], pb[2][:, 2:4], ADD
        )
        for bi in range(3, len(BLOCKS)):
            nc.gpsimd.tensor_tensor(xb[bi][:], fb[bi][:], pb[bi][:], ADD)

        # ---- x^T via SBUF->SBUF DMA transpose (out[p, j, c] = x[c, 128j+p]) ----
        xt = []
        for bi, (r0, nr) in enumerate(BLOCKS):
            t = xtp.tile([128, 2 * nr, 128], F16, tag=f"xt{r0}", name=f"xt{r0}")
            nc.sync.dma_start_transpose(t[:], xb[bi][:])
            xt.append(t)

        def xt_chunk(g, c):
            # global row g, W-chunk c -> (tile, chunk index)
            for bi, (r0, nr) in enumerate(BLOCKS):
                if r0 <= g < r0 + nr:
                    return xt[bi][:, 2 * (g - r0) + c, :]
            raise AssertionError

        wq_t = cb[:, 0:128]
        wk_t = cb[:, 128:256]
        ident = cb[:, 256:384]
        ones_t = cb[:, 384:512]
        mask_m = cb[:, 512:784]       # [C, 2, 136] additive: 0 valid, -30 banned
        oob_m = cb[:, 784:816]        # [C, 2, 2, 8] edge phantom-tap counts * e^-6
        bq_t = cf[:, 0:1]
        bk_t = cf[:, 1:2]
        nsh_t = cf[:, 2:3]

        # ---- PE warm-up: keep the HAM clock gate busy during the ramp ----
        wps = pss.tile([C, 2, 2, 256], F32, tag="s")
        for _ in range(WARMN):
            nc.tensor.matmul(wps[:, 0, 0, 0:128], dm[:], dm[:], start=True, stop=True)

        NIT = ROWS // 2
        qsb = {}
        ksb = {}
        sps = {}
        att = {}
        dps = {}
        rdn = {}
        ops = {}
        osb = {}

        def qk_mm(si):
            q_ps = psq.tile([C, 2, 512], F32, tag="q")
            k_ps = psk.tile([C, 2, 512], F32, tag="k")
            if si == 0:
                # per-half so the first scores start as soon as block0 lands
                nc.tensor.matmul(q_ps[:, 0], wq_t, apn(xb[0], [(1, 512)]), start=True, stop=True)
                nc.tensor.matmul(k_ps[:, 0], wk_t, apn(xb[0], [(1, 512)]), start=True, stop=True)
                nc.tensor.matmul(q_ps[:, 1], wq_t, apn(xb[1], [(1, 512)]), start=True, stop=True)
                nc.tensor.matmul(k_ps[:, 1], wk_t, apn(xb[1], [(1, 512)]), start=True, stop=True)
            else:
                src = xb[si + 1]
                for hh in range(2):
                    nc.tensor.matmul(
                        q_ps[:, hh], wq_t,
                        apn(src, [(1, 512)], off=hh * 512),
                        start=True, stop=True,
                    )
                for hh in range(2):
                    nc.tensor.matmul(
                        k_ps[:, hh], wk_t,
                        apn(src, [(1, 512)], off=hh * 512),
                        start=True, stop=True,
                    )
            qsb[si] = qkp.tile([C, 2, 512], F16, tag="q", name="qsb")
            ksb[si] = qkp.tile([C, 2, 512], F16, tag="k", name="ksb")
            return q_ps, k_ps

        def ev_qk(si, q_ps, k_ps):
            if si == 0:
                # halves in parallel across scalar/vector for ramp latency
                nc.vector.tensor_scalar_add(qsb[0][:, 0], q_ps[:, 0], bq_t)
                nc.scalar.add(ksb[0][:, 0], k_ps[:, 0], bk_t)
                nc.scalar.add(qsb[0][:, 1], q_ps[:, 1], bq_t)
                nc.vector.tensor_scalar_add(ksb[0][:, 1], k_ps[:, 1], bk_t)
            else:
                nc.scalar.add(ksb[si][:], k_ps[:], bk_t)
                nc.scalar.add(qsb[si][:, 0], q_ps[:, 0], bq_t)
                nc.vector.tensor_scalar_add(qsb[si][:, 1], q_ps[:, 1], bq_t)

        def stageA(it):
            si, h = divmod(it, 2)
            q_sb = qsb[si][:, h]
            k_sb = ksb[si][:, h]
            s_ps = pss.tile([C, 2, 2, 256], F32, tag="s")
            for rr in range(2):
                q0 = rr * 256
                nc.tensor.matmul(
                    s_ps[:, rr, 0, 0:SW], k_sb[:, q0 : q0 + 128], q_sb[:, q0 : q0 + SW],
                    start=True, stop=False,
                )
                # additive band mask (-30 banned / 0 valid), accumulated pre-exp.
                # Each accumulate immediately follows its region's start=True
                # write (no intervening start=True elsewhere in the bank).
                nc.tensor.matmul(
                    s_ps[:, rr, 0, 0:SW],
                    ident, apn(cb, [(1, SW)], off=512),
                    start=False, stop=False, skip_group_check=True,
                )
                nc.tensor.matmul(
                    s_ps[:, rr, 1, 0:SW], k_sb[:, q0 + 128 : q0 + 256], q_sb[:, q0 + 120 : q0 + 256],
                    start=True, stop=False,
                )
                nc.tensor.matmul(
                    s_ps[:, rr, 1, 0:SW],
                    ident, apn(cb, [(1, SW)], off=512 + SW),
                    start=False, stop=(rr == 1), skip_group_check=True,
                )
            sps[it] = s_ps

        def stageExp(it):
            a = attp.tile([C, 2, 2, SW], F16)
            nc.scalar.activation(
                a[:], apn(sps.pop(it), [(512, 2), (256, 2), (1, SW)]), EXP,
                bias=nsh_t,
            )
            att[it] = a

        def stageB(it):
            # denominators, broadcast across partitions by the ones matmul
            d = psd.tile([C, 2, 256], F32, tag="d")
            dps[it] = d
            a = att[it]
            nc.tensor.matmul(
                apn(d, [(256, 2), (1, SW)]),
                ones_t, apn(a, [(272, 2), (1, SW)]),
                start=True, stop=False,
            )
            nc.tensor.matmul(
                apn(d, [(256, 2), (1, 16)], off=120),
                ones_t, apn(a, [(272, 2), (1, 16)], off=SW),
                start=False, stop=False, skip_group_check=True,
            )
            # zero-pad phantom taps (exp(0-SHIFT) each) only exist at the edges;
            # left edge accumulates BEFORE the next start=True in this bank
            nc.tensor.matmul(
                apn(d, [(256, 2), (1, 8)]),
                ident, apn(cb, [(16, 2), (1, 8)], off=784),
                start=False, stop=False, skip_group_check=True,
            )
            nc.tensor.matmul(
                apn(d, [(256, 2), (1, 120)], off=SW),
                ones_t, apn(a, [(272, 2), (1, 120)], off=SW + 16),
                start=True, stop=False, skip_group_check=True,
            )
            nc.tensor.matmul(
                apn(d, [(256, 2), (1, 8)], off=248),
                ident, apn(cb, [(16, 2), (1, 8)], off=784 + 8),
                start=False, stop=True, skip_group_check=True,
            )

        def recip(it):
            r = rdp.tile([C, 512], F32)
            nc.vector.reciprocal_approx_fast(out=r[:], in_=apn(dps.pop(it), [(1, 512)]))
            rdn[it] = r

        def stageC(it):
            o = pso.tile([C, 2, 256], F32, tag="o")
            ops[it] = o
            a = att.pop(it)
            for rr in range(2):
                g = 2 * it + rr
                o0 = rr * 256
                nc.tensor.matmul(
                    apn(o, [(1, SW)], off=o0),
                    xt_chunk(g, 0), a[:, rr, 0, :],
                    start=True, stop=False,
                )
                nc.tensor.matmul(
                    apn(o, [(1, 16)], off=o0 + 120),
                    xt_chunk(g, 1), a[:, rr, 1, 0:16],
                    start=False, stop=True, skip_group_check=True,
                )
                nc.tensor.matmul(
                    apn(o, [(1, 120)], off=o0 + SW),
                    xt_chunk(g, 1), a[:, rr, 1, 16:SW],
                    start=True, stop=True, skip_group_check=True,
                )

        def norm(it):
            si, h = divmod(it, 2)
            if h == 0:
                osb[si] = osp.tile([C, 2, 512], F16, tag="osb", name="osb")
            nc.vector.tensor_tensor(
                osb[si][:, h], apn(ops.pop(it), [(1, 512)]), rdn.pop(it)[:],
                MULT,
            )

        def out_dma(si):
            nc.sync.dma_start(out_ext[:, 4 * si : 4 * si + 4, :], osb[si][:])

        # ---- software pipeline ----
        q0, k0 = qk_mm(0)
        ev_qk(0, q0, k0)
        stageA(0)
        stageExp(0)
        stageA(1)
        stageExp(1)
        stageB(0)
        recip(0)
        stageB(1)
        recip(1)
        for si in range(1, 4):
            i0 = 2 * si
            q_, k_ = qk_mm(si)
            ev_qk(si, q_, k_)
            stageA(i0)
            stageExp(i0)
            stageC(i0 - 2)
            norm(i0 - 2)
            stageA(i0 + 1)
            stageExp(i0 + 1)
            stageC(i0 - 1)
            norm(i0 - 1)
            stageB(i0)
            recip(i0)
            stageB(i0 + 1)
            recip(i0 + 1)
            out_dma(si - 1)
        stageC(6)
        norm(6)
        stageC(7)
        norm(7)
        out_dma(3)

    nc.compile()
    return nc


def host_consts(Wq, bq, Wk, bk):
    sc = 1.0 / np.sqrt(np.float32(C))
    wqt = np.ascontiguousarray(Wq.astype(np.float32).T * sc)
    wkt = np.ascontiguousarray(Wk.astype(np.float32).T)
    ident = np.eye(C, dtype=np.float32)
    ones = np.ones((C, C), dtype=np.float32)

    # additive band mask on the two strips (same for both rows of an iter):
    # chunk1: key p vs query w in [0, SW); chunk2: key 128+p vs query 120+w
    maskm = np.full((C, 2, SW), -30.0, dtype=np.float32)
    for p in range(C):
        for w in range(SW):
            if abs(p - w) <= R:
                maskm[p, 0, w] = 0.0
            if abs(p + 8 - w) <= R:
                maskm[p, 1, w] = 0.0
    maskm = maskm.reshape(C, 2 * SW)

    # zero-pad phantom-tap counts at the edges, pre-scaled by e^-SHIFT
    # (the ident-matmul adds these per element; only w<8 / w>247 are nonzero)
    es = np.exp(-SHIFT)
    left = np.array([(R - j) * es for j in range(8)], dtype=np.float32)
    right = np.array([(j + 1) * es for j in range(8)], dtype=np.float32)
    oob = np.concatenate([left, right])          # [2, 8] -> 16 per row
    oobm = np.tile(oob, (C, 2)).astype(np.float32)  # [C, 2 rows, 16]
    assert oobm.shape == (C, 32)

    constb = np.concatenate(
        [wqt, wkt, ident, ones, maskm, oobm], axis=1
    ).astype(np.float16)
    assert constb.shape == (C, CBW), constb.shape
    constf = np.stack(
        [
            bq.astype(np.float32) * sc,
            bk.astype(np.float32),
            np.full(C, -SHIFT, dtype=np.float32),
        ],
        axis=1,
    ).reshape(C, 3)
    return np.ascontiguousarray(constb), np.ascontiguousarray(constf)


def core_inputs(feature, position, Wq, bq, Wk, bk):
    constb, constf = host_consts(Wq, bq, Wk, bk)
    in_maps = []
    for i in range(NCORES):
        b = i // CORES_PER_B
        h0 = (i % CORES_PER_B) * ROWS
        in_maps.append(
            {
                "feature": np.ascontiguousarray(
                    feature[b, :, h0 : h0 + ROWS, :], dtype=np.float16
                ),
                "position": np.ascontiguousarray(
                    position[b, :, h0 : h0 + ROWS, :], dtype=np.float16
                ),
                "constb": constb,
                "constf": constf,
            }
        )
    return in_maps


def kernel(feature, position, Wq, bq, Wk, bk):
    feature = np.asarray(feature, dtype=np.float32)
    position = np.asarray(position, dtype=np.float32)
    Wq = np.asarray(Wq, dtype=np.float32)
    bq = np.asarray(bq, dtype=np.float32)
    Wk = np.asarray(Wk, dtype=np.float32)
    bk = np.asarray(bk, dtype=np.float32)
    in_maps = core_inputs(feature, position, Wq, bq, Wk, bk)
    nc = build_nc()
    res = run_bass_kernel_spmd(nc, in_maps, list(range(NCORES)))
    out = np.empty((B, C, H, W), dtype=np.float32)
    for i in range(NCORES):
        b = i // CORES_PER_B
        h0 = (i % CORES_PER_B) * ROWS
        out[b, :, h0 : h0 + ROWS, :] = res.results[i]["out"].astype(np.float32)
    return out
